# revision 17
# baseline (speedup 1.0000x reference)
"""NT-Xent contrastive loss on 8 Trainium2 NeuronCores (Bass/Tile), fp8.

Strategy (no collectives; ncfw collective latency floor ~85us):
  * Host casts embT to fp8e4 [2048, 8192] (sigma=1 fits e4m3) and W*64 to
    fp8e4; b*64 stays f32.  Slab cover: core c loads the 4 column-slabs
    S_c = {c, c+1, c+2, c+4} (mod 8) of embT (8.4 MB/core).  Every slab
    pair meets on some core (Z8 difference cover), so each distinct
    1024x1024 block of the 8192x8192 similarity matrix is computed once
    globally (the diff-4 block is deduped on host: cores 0-3 win).
  * Per core: head matmul in fp8 DoubleRow (K=256/instr, 0.5 cyc/row)
    -> h' = 64h in psum -> bias-add copy to bf16 (Pool dh0 / DVE dh1).
    L2 norm: nsq via bf16 ones-matmul into a [33,512] psum tile (rows 0
    and 32), then r = exp(-0.5*ln(nsq) + ln8) on ACT (ln+exp share one
    activation table with the sim exp => zero table reloads), broadcast
    down partitions with gpsimd partition_broadcast, t_on = h*r in fp8e4
    (= 8 * normalized out).
  * 5 sim blocks/core (diag + 4 pairs): one DoubleRow matmul per
    [128,1024] psum tile; diag killed pre-exp with an additive -1e9
    shifted mask (DVE); ACT exp(0.15625*x) with fused row-sum accum
    writes fp8e5 exp values; column sums via DoubleRow ones-matmul over
    mb-pair-interleaved e5 tiles at the end.
  * pos: bf16 product of t_h slabs 0,3 + ones-matmul + r-scales; host
    divides by 64.  Host combine in fp64.
"""
import math
import numpy as np
import ml_dtypes

SLOTS = [(c, (c + 1) % 8, (c + 2) % 8, (c + 4) % 8) for c in range(8)]
# sim units: (stationary slot, moving slot, e5 colsum slot or None)
UNITS = [(0, 0, None), (0, 1, 0), (0, 2, 1), (0, 3, 2), (1, 3, 3)]
LN8 = math.log(8.0)

_CACHE = {}


def _build():
    if "nc" in _CACHE:
        return _CACHE["nc"]
    import concourse.bacc as bacc
    import concourse.tile as tile
    import concourse.mybir as mybir

    F32 = mybir.dt.float32
    BF16 = mybir.dt.bfloat16
    E4 = mybir.dt.float8e4
    E5 = mybir.dt.float8e5
    AF = mybir.ActivationFunctionType
    ALU = mybir.AluOpType
    DR = mybir.MatmulPerfMode.DoubleRow

    nc = bacc.Bacc("TRN2", num_devices=8, debug=False)
    a_emb = nc.dram_tensor("embT8", [2048, 4096], E4, kind="ExternalInput").ap()
    a_W = nc.dram_tensor("W8", [2048, 256], E4, kind="ExternalInput").ap()
    a_b = nc.dram_tensor("b64", [256], F32, kind="ExternalInput").ap()
    a_o1 = nc.dram_tensor("onesbf", [128, 1], BF16, kind="ExternalInput").ap()
    a_o5 = nc.dram_tensor("ones5", [128, 256], E5, kind="ExternalInput").ap()
    a_mask = nc.dram_tensor("mask", [128, 2048], E4, kind="ExternalInput").ap()
    a_id = nc.dram_tensor("ident", [128, 128], E4, kind="ExternalInput").ap()
    o_rp = nc.dram_tensor("rowpart", [128, 40], F32, kind="ExternalOutput").ap()
    o_cp = nc.dram_tensor("colpart", [1, 4096], F32, kind="ExternalOutput").ap()
    o_ps = nc.dram_tensor("possim", [1, 1024], F32, kind="ExternalOutput").ap()

    with tile.TileContext(nc) as tc:
        with tc.tile_pool(name="sb", bufs=1) as sb, \
             tc.tile_pool(name="wk", bufs=2) as wk, \
             tc.tile_pool(name="hp", bufs=2, space="PSUM") as hp, \
             tc.tile_pool(name="simp", bufs=2, space="PSUM") as simp, \
             tc.tile_pool(name="smp", bufs=1, space="PSUM") as smp:

            # ---- persistent tiles + prologue DMAs.  Critical path first:
            # t_W then stage-0 emb tiles on the sync queue; everything else
            # (consts, stages 1-3) on the gpsimd queue in parallel.
            t_W = sb.tile([128, 8, 2, 2, 128], E4, name="t_W")
            nc.sync.dma_start(
                t_W[:],
                a_W.rearrange("(kk j p) (dh f) -> p kk j dh f",
                              kk=8, j=2, p=128, dh=2, f=128))
            t_e8 = [[None] * 8 for _ in range(4)]
            def load_emb(k, kk, eng):
                t = sb.tile([128, 2, 1024], E4, name=f"t_e8_{k}_{kk}")
                esrc = a_emb[256 * kk:256 * (kk + 1),
                             1024 * k:1024 * (k + 1)]
                eng.dma_start(t[:], esrc.rearrange("(j p) s -> p j s",
                                                   j=2, p=128))
                t_e8[k][kk] = t
            for kk in range(8):
                load_emb(0, kk, nc.sync)
            t_b = sb.tile([128, 2], F32, name="t_b")
            nc.gpsimd.dma_start(t_b[:], a_b.rearrange("(dh p) -> p dh",
                                                      p=128))
            t_o1 = sb.tile([128, 1], BF16, name="t_o1")
            nc.gpsimd.dma_start(t_o1[:], a_o1[:])
            t_o5 = sb.tile([128, 2, 128], E5, name="t_o5")
            nc.gpsimd.dma_start(t_o5[:], a_o5.rearrange("p (j f) -> p j f",
                                                        j=2, f=128))
            t_id = sb.tile([128, 128], E4, name="t_id")
            nc.gpsimd.dma_start(t_id[:], a_id[:])
            t_mask = sb.tile([128, 2048], E4, name="t_mask")
            nc.gpsimd.dma_start(t_mask[:], a_mask[:])
            for kk in range(8):
                load_emb(1, kk, nc.sync)
            for kk in range(8):
                load_emb(2, kk, nc.gpsimd)
            for kk in range(8):
                load_emb(3, kk, nc.sync)

            t_h = [sb.tile([128, 2, 1024], BF16, name=f"t_h{k}")
                   for k in range(4)]
            t_r_tiles = [None] * 4
            t_on = sb.tile([128, 2, 4, 1024], E4, name="t_on")
            t_e5 = sb.tile([128, 8, 4, 1024], E5, name="t_e5")
            t_scr = sb.tile([128, 1024], E5, name="t_scr")
            rp_st = sb.tile([128, 40], F32, name="rp_st")
            cp_st = sb.tile([1, 4096], F32, name="cp_st")
            ps_st = sb.tile([1, 1024], F32, name="ps_st")

            def head_chain(k, dh, h):
                H = hp.tile([128, 512], F32, name=f"H{k}_{dh}_{h}", tag="H")
                for kk in range(8):
                    nc.tensor.matmul(
                        H[:], t_W[:, kk, :, dh, :],
                        t_e8[k][kk][:, :, 512 * h:512 * (h + 1)],
                        start=(kk == 0), stop=(kk == 7), perf_mode=DR)
                nc.vector.tensor_scalar_add(
                    t_h[k][:, dh, 512 * h:512 * (h + 1)], H[:],
                    t_b[:, dh:dh + 1])

            def norm(k):
                t_sq = wk.tile([128, 2, 1024], BF16, name="t_sq", tag="sq")
                nc.vector.tensor_tensor(t_sq[:], t_h[k][:], t_h[k][:],
                                        ALU.mult)
                # t_o1 holds 1/64, so nsq psum = nsq'/64 and
                # exp(-0.5*ln(x)) = 8/sqrt(nsq') -- no activation bias needed
                r_bc = wk.tile([128, 1024], F32, name="r_bc", tag="rbc")
                nsq = smp.tile([1, 1024], F32, name=f"nsq{k}", tag="sm")
                for nh in range(2):
                    for dh in range(2):
                        nc.tensor.matmul(
                            nsq[0:1, 512 * nh:512 * (nh + 1)], t_o1[:],
                            t_sq[:, dh, 512 * nh:512 * (nh + 1)],
                            start=(dh == 0), stop=(dh == 1))
                nln = wk.tile([1, 1024], F32, name="nln", tag="nln")
                nc.scalar.activation(nln[:], nsq[:], AF.Ln)
                t_rk = sb.tile([1, 1024], F32, name=f"t_r{k}")
                t_r_tiles[k] = t_rk
                nc.scalar.activation(t_rk[:], nln[:], AF.Exp, scale=-0.5)
                for nh in range(2):
                    nc.gpsimd.partition_broadcast(
                        r_bc[:, 512 * nh:512 * (nh + 1)],
                        t_rk[0:1, 512 * nh:512 * (nh + 1)])
                for dh in range(2):
                    nc.vector.tensor_tensor(t_on[:, dh, k, :],
                                            t_h[k][:, dh, :], r_bc[:],
                                            ALU.mult)

            def unit(u, a, b, e5slot, mb):
                ps = simp.tile([128, 1024], F32, name="ps", tag="ps")
                diag = a == b
                for nb in range(2):
                    nc.tensor.matmul(ps[:, 512 * nb:512 * (nb + 1)],
                                     t_on[:, :, a, 128 * mb:128 * (mb + 1)],
                                     t_on[:, :, b, 512 * nb:512 * (nb + 1)],
                                     start=True, stop=not diag,
                                     perf_mode=DR)
                    if diag:
                        # accumulate 240*I @ mask(-4 at diag) = -960 on the
                        # self-similarity entries; exp flushes them to 0
                        nc.tensor.matmul(
                            ps[:, 512 * nb:512 * (nb + 1)], t_id[:],
                            t_mask[:, 1024 - 128 * mb + 512 * nb:
                                   1024 - 128 * mb + 512 * (nb + 1)],
                            start=False, stop=True)
                dest = t_scr[:] if e5slot is None else t_e5[:, mb, e5slot, :]
                nc.scalar.activation(dest, ps[:], AF.Exp, scale=0.15625,
                                     accum_out=rp_st[:, u * 8 + mb:
                                                     u * 8 + mb + 1])

            def stage_full(k):
                for dh in range(2):
                    for h in range(2):
                        head_chain(k, dh, h)
                norm(k)

            def colsums(ci):
                # column sums for e5 slot ci (DoubleRow over mb pairs)
                for nh in range(2):
                    cs = smp.tile([128, 512], F32, name=f"cs{ci}_{nh}",
                                  tag="sm")
                    for jj in range(4):
                        nc.tensor.matmul(
                            cs[:], t_o5[:],
                            t_e5[:, 2 * jj:2 * jj + 2, ci,
                                 512 * nh:512 * (nh + 1)],
                            start=(jj == 0), stop=(jj == 3), perf_mode=DR)
                    nc.vector.tensor_copy(
                        cp_st[0:1, 1024 * ci + 512 * nh:
                              1024 * ci + 512 * (nh + 1)], cs[0:1, :])

            def emit_unit(u, mb):
                unit(u, *UNITS[u][:2], UNITS[u][2], mb)
                if mb == 7 and UNITS[u][2] is not None:
                    colsums(UNITS[u][2])

            stage_full(0)
            for k in range(1, 4):
                pu = k - 1
                emit_unit(pu, 0)
                emit_unit(pu, 1)
                head_chain(k, 0, 0)
                emit_unit(pu, 2)
                emit_unit(pu, 3)
                head_chain(k, 0, 1)
                emit_unit(pu, 4)
                emit_unit(pu, 5)
                head_chain(k, 1, 0)
                emit_unit(pu, 6)
                emit_unit(pu, 7)
                head_chain(k, 1, 1)
                norm(k)
            for mb in range(8):
                emit_unit(3, mb)
            for mb in range(8):
                emit_unit(4, mb)

            # pos: bf16 product of t_h slabs 0 and 3, ones-matmul, r-scales
            t_pp = wk.tile([128, 2, 1024], BF16, name="t_pp", tag="sq")
            nc.vector.tensor_tensor(t_pp[:], t_h[0][:], t_h[3][:], ALU.mult)
            pr = smp.tile([1, 1024], F32, name="rawdot", tag="sm")
            for nh in range(2):
                for dh in range(2):
                    nc.tensor.matmul(pr[0:1, 512 * nh:512 * (nh + 1)],
                                     t_o1[:],
                                     t_pp[:, dh, 512 * nh:512 * (nh + 1)],
                                     start=(dh == 0), stop=(dh == 1))
            tmp = wk.tile([1, 1024], F32, name="ptmp", tag="nln")
            nc.vector.tensor_tensor(tmp[:], pr[:], t_r_tiles[0][:], ALU.mult)
            nc.vector.tensor_tensor(ps_st[:], tmp[:], t_r_tiles[3][:],
                                    ALU.mult)


            nc.sync.dma_start(o_rp, rp_st[:])
            nc.sync.dma_start(o_cp, cp_st[:])
            nc.sync.dma_start(o_ps, ps_st[:])

    # Keep Exp/Ln selectable only from the single table set that holds both,
    # so the compiler never ping-pongs ACT table loads between exp-only and
    # ln-only sets (1283ns per reload).  Entries stay in place so
    # act_func_set_id indices still match act_info.json.
    import concourse.bacc as bacc_mod
    orig_get = bacc_mod.get_activation_tables

    def _pinned_tables(arch):
        tabs = orig_get(arch)
        AFT = mybir.ActivationFunctionType
        both = [k for k, v in tabs.items() if AFT.Exp in v and AFT.Ln in v]
        if not both:
            return tabs
        keep = both[0]
        out = {}
        for k, v in tabs.items():
            if k == keep:
                out[k] = v
            else:
                out[k] = {f for f in v if f not in (AFT.Exp, AFT.Ln)}
        return out

    bacc_mod.get_activation_tables = _pinned_tables
    try:
        nc.compile()
    finally:
        bacc_mod.get_activation_tables = orig_get
    _CACHE["nc"] = nc
    return nc


def _host_inputs(embedded_data, W, b):
    E4np = ml_dtypes.float8_e4m3
    E5np = ml_dtypes.float8_e5m2
    emb = np.asarray(embedded_data, dtype=np.float32)
    embT8 = np.ascontiguousarray(emb.T).astype(E4np)      # [2048, 8192]
    W8 = (np.asarray(W, dtype=np.float32) * 64.0).astype(E4np)
    b64 = (np.asarray(b, dtype=np.float32) * 64.0).astype(np.float32)
    o1 = np.full((128, 1), 1.0 / 64.0, ml_dtypes.bfloat16)
    o5 = np.ones((128, 256), E5np)
    mask = np.zeros((128, 2048), E4np)
    mask[np.arange(128), np.arange(128) + 1024] = -4.0
    ident = (240.0 * np.eye(128, dtype=np.float32)).astype(E4np)
    in_maps = []
    for c in range(8):
        cols = np.concatenate(
            [embT8[:, 1024 * s:1024 * (s + 1)] for s in SLOTS[c]], axis=1)
        in_maps.append({"embT8": np.ascontiguousarray(cols), "W8": W8,
                        "b64": b64, "onesbf": o1, "ones5": o5, "mask": mask,
                        "ident": ident})
    return in_maps


def _combine(results):
    neg = np.zeros(8192, np.float64)
    pos = np.zeros(8192, np.float64)
    for c in range(8):
        S = SLOTS[c]
        rp = results[c]["rowpart"].astype(np.float64)     # [128, 40]
        cp = results[c]["colpart"].astype(np.float64).ravel()
        ps = results[c]["possim"].astype(np.float64)
        sl = [np.s_[1024 * s:1024 * (s + 1)] for s in S]
        for u, (astat, _, _) in enumerate(UNITS):
            if u == 3 and c >= 4:
                continue                                   # diff-4 dedup
            dst = 1024 * S[astat]
            for mb in range(8):
                neg[dst + 128 * mb:dst + 128 * (mb + 1)] += rp[:, 8 * u + mb]
        neg[sl[1]] += cp[0:1024]
        neg[sl[2]] += cp[1024:2048]
        if c < 4:
            neg[sl[3]] += cp[2048:3072]
        neg[sl[3]] += cp[3072:4096]
        if c < 4:
            possim = ps.ravel()
            pos[sl[0]] = possim
            pos[sl[3]] = possim
    loss = -np.mean(10.0 * pos - np.log(neg))
    return np.float32(loss)


def run(embedded_data, W, b, trace=False):
    from concourse import bass_utils
    nc = _build()
    in_maps = _host_inputs(embedded_data, W, b)
    res = bass_utils.run_bass_kernel_spmd(nc, in_maps, core_ids=list(range(8)),
                                          trace=trace)
    return _combine(res.results), res


def kernel(embedded_data, W, b):
    loss, _ = run(embedded_data, W, b, trace=False)
    return np.asarray(loss, dtype=np.float32)


# revision 18
# speedup vs baseline: 1.0645x; 1.0645x over previous
"""NT-Xent contrastive loss on 8 Trainium2 NeuronCores (Bass/Tile), fp8.

Strategy (no collectives; ncfw collective latency floor ~85us):
  * Host casts embT to fp8e4 [2048, 8192] (sigma=1 fits e4m3) and W*64 to
    fp8e4; b*64 stays f32.  Slab cover: core c loads the 4 column-slabs
    S_c = {c, c+1, c+2, c+4} (mod 8) of embT (8.4 MB/core).  Every slab
    pair meets on some core (Z8 difference cover), so each distinct
    1024x1024 block of the 8192x8192 similarity matrix is computed once
    globally (the diff-4 block is deduped on host: cores 0-3 win).
  * Per core: head matmul in fp8 DoubleRow (K=256/instr, 0.5 cyc/row)
    -> h' = 64h in psum -> bias-add copy to bf16 (Pool dh0 / DVE dh1).
    L2 norm: nsq via bf16 ones-matmul into a [33,512] psum tile (rows 0
    and 32), then r = exp(-0.5*ln(nsq) + ln8) on ACT (ln+exp share one
    activation table with the sim exp => zero table reloads), broadcast
    down partitions with gpsimd partition_broadcast, t_on = h*r in fp8e4
    (= 8 * normalized out).
  * 5 sim blocks/core (diag + 4 pairs): one DoubleRow matmul per
    [128,1024] psum tile; diag killed pre-exp with an additive -1e9
    shifted mask (DVE); ACT exp(0.15625*x) with fused row-sum accum
    writes fp8e5 exp values; column sums via DoubleRow ones-matmul over
    mb-pair-interleaved e5 tiles at the end.
  * pos: bf16 product of t_h slabs 0,3 + ones-matmul + r-scales; host
    divides by 64.  Host combine in fp64.
"""
import math
import numpy as np
import ml_dtypes

SLOTS = [(c, (c + 1) % 8, (c + 2) % 8, (c + 4) % 8) for c in range(8)]
# sim units: (stationary slot, moving slot, e5 colsum slot or None)
UNITS = [(0, 0, None), (0, 1, 0), (0, 2, 1), (0, 3, 2), (1, 3, 3)]
LN8 = math.log(8.0)

_CACHE = {}


def _build():
    if "nc" in _CACHE:
        return _CACHE["nc"]
    import concourse.bacc as bacc
    import concourse.tile as tile
    import concourse.mybir as mybir

    F32 = mybir.dt.float32
    BF16 = mybir.dt.bfloat16
    E4 = mybir.dt.float8e4
    E5 = mybir.dt.float8e5
    AF = mybir.ActivationFunctionType
    ALU = mybir.AluOpType
    DR = mybir.MatmulPerfMode.DoubleRow

    nc = bacc.Bacc("TRN2", num_devices=8, debug=False)
    a_emb = nc.dram_tensor("embT8", [2048, 4096], E4, kind="ExternalInput").ap()
    a_W = nc.dram_tensor("W8", [2048, 256], E4, kind="ExternalInput").ap()
    a_b = nc.dram_tensor("b64", [256], F32, kind="ExternalInput").ap()
    a_o1 = nc.dram_tensor("onesbf", [128, 1], BF16, kind="ExternalInput").ap()
    a_o5 = nc.dram_tensor("ones5", [128, 256], E5, kind="ExternalInput").ap()
    a_mask = nc.dram_tensor("mask", [128, 2048], E4, kind="ExternalInput").ap()
    a_id = nc.dram_tensor("ident", [128, 128], E4, kind="ExternalInput").ap()
    o_rp = nc.dram_tensor("rowpart", [128, 40], F32, kind="ExternalOutput").ap()
    o_cp = nc.dram_tensor("colpart", [1, 4096], F32, kind="ExternalOutput").ap()
    o_ps = nc.dram_tensor("possim", [1, 1024], F32, kind="ExternalOutput").ap()

    with tile.TileContext(nc) as tc:
        with tc.tile_pool(name="sb", bufs=1) as sb, \
             tc.tile_pool(name="wk", bufs=2) as wk, \
             tc.tile_pool(name="hp", bufs=2, space="PSUM") as hp, \
             tc.tile_pool(name="simp", bufs=2, space="PSUM") as simp, \
             tc.tile_pool(name="smp", bufs=1, space="PSUM") as smp:

            # ---- persistent tiles + prologue DMAs.  Critical path first:
            # t_W then stage-0 emb tiles on the sync queue; everything else
            # (consts, stages 1-3) on the gpsimd queue in parallel.
            t_W = sb.tile([128, 8, 2, 2, 128], E4, name="t_W")
            nc.sync.dma_start(
                t_W[:],
                a_W.rearrange("(kk j p) (dh f) -> p kk j dh f",
                              kk=8, j=2, p=128, dh=2, f=128))
            t_e8 = [[None] * 8 for _ in range(4)]
            def load_emb(k, kk, eng):
                t = sb.tile([128, 2, 1024], E4, name=f"t_e8_{k}_{kk}")
                esrc = a_emb[256 * kk:256 * (kk + 1),
                             1024 * k:1024 * (k + 1)]
                eng.dma_start(t[:], esrc.rearrange("(j p) s -> p j s",
                                                   j=2, p=128))
                t_e8[k][kk] = t
            for kk in range(8):
                load_emb(0, kk, nc.sync)
            t_b = sb.tile([128, 2], F32, name="t_b")
            nc.sync.dma_start(t_b[:], a_b.rearrange("(dh p) -> p dh",
                                                    p=128))
            t_o1 = sb.tile([128, 1], BF16, name="t_o1")
            nc.sync.dma_start(t_o1[:], a_o1[:])
            t_o5 = sb.tile([128, 2, 128], E5, name="t_o5")
            nc.sync.dma_start(t_o5[:], a_o5.rearrange("p (j f) -> p j f",
                                                      j=2, f=128))
            t_id = sb.tile([128, 128], E4, name="t_id")
            nc.sync.dma_start(t_id[:], a_id[:])
            t_mask = sb.tile([128, 2048], E4, name="t_mask")
            nc.sync.dma_start(t_mask[:], a_mask[:])
            for kk in range(8):
                load_emb(1, kk, nc.sync)
            for kk in range(8):
                load_emb(2, kk, nc.sync)
            for kk in range(8):
                load_emb(3, kk, nc.sync)

            t_h = [sb.tile([128, 2, 1024], BF16, name=f"t_h{k}")
                   for k in range(4)]
            t_r_tiles = [None] * 4
            t_on = sb.tile([128, 2, 4, 1024], E4, name="t_on")
            t_e5 = sb.tile([128, 8, 4, 1024], E5, name="t_e5")
            t_scr = sb.tile([128, 1024], E5, name="t_scr")
            rp_st = sb.tile([128, 40], F32, name="rp_st")
            cp_st = sb.tile([1, 4096], F32, name="cp_st")
            ps_st = sb.tile([1, 1024], F32, name="ps_st")

            def head_chain(k, dh, h):
                H = hp.tile([128, 512], F32, name=f"H{k}_{dh}_{h}", tag="H")
                for kk in range(8):
                    nc.tensor.matmul(
                        H[:], t_W[:, kk, :, dh, :],
                        t_e8[k][kk][:, :, 512 * h:512 * (h + 1)],
                        start=(kk == 0), stop=(kk == 7), perf_mode=DR)
                nc.vector.tensor_scalar_add(
                    t_h[k][:, dh, 512 * h:512 * (h + 1)], H[:],
                    t_b[:, dh:dh + 1])

            def norm(k):
                t_sq = wk.tile([128, 2, 1024], BF16, name="t_sq", tag="sq")
                nc.vector.tensor_tensor(t_sq[:], t_h[k][:], t_h[k][:],
                                        ALU.mult)
                # t_o1 holds 1/64, so nsq psum = nsq'/64 and
                # exp(-0.5*ln(x)) = 8/sqrt(nsq') -- no activation bias needed
                r_bc = wk.tile([128, 1024], F32, name="r_bc", tag="rbc")
                nsq = smp.tile([1, 1024], F32, name=f"nsq{k}", tag="sm")
                for nh in range(2):
                    for dh in range(2):
                        nc.tensor.matmul(
                            nsq[0:1, 512 * nh:512 * (nh + 1)], t_o1[:],
                            t_sq[:, dh, 512 * nh:512 * (nh + 1)],
                            start=(dh == 0), stop=(dh == 1))
                nln = wk.tile([1, 1024], F32, name="nln", tag="nln")
                nc.scalar.activation(nln[:], nsq[:], AF.Ln)
                t_rk = sb.tile([1, 1024], F32, name=f"t_r{k}")
                t_r_tiles[k] = t_rk
                nc.scalar.activation(t_rk[:], nln[:], AF.Exp, scale=-0.5)
                for nh in range(2):
                    nc.gpsimd.partition_broadcast(
                        r_bc[:, 512 * nh:512 * (nh + 1)],
                        t_rk[0:1, 512 * nh:512 * (nh + 1)])
                for dh in range(2):
                    nc.vector.tensor_tensor(t_on[:, dh, k, :],
                                            t_h[k][:, dh, :], r_bc[:],
                                            ALU.mult)

            def unit(u, a, b, e5slot, mb):
                ps = simp.tile([128, 1024], F32, name="ps", tag="ps")
                diag = a == b
                for nb in range(2):
                    nc.tensor.matmul(ps[:, 512 * nb:512 * (nb + 1)],
                                     t_on[:, :, a, 128 * mb:128 * (mb + 1)],
                                     t_on[:, :, b, 512 * nb:512 * (nb + 1)],
                                     start=True, stop=not diag,
                                     perf_mode=DR)
                    if diag:
                        # accumulate 240*I @ mask(-4 at diag) = -960 on the
                        # self-similarity entries; exp flushes them to 0
                        nc.tensor.matmul(
                            ps[:, 512 * nb:512 * (nb + 1)], t_id[:],
                            t_mask[:, 1024 - 128 * mb + 512 * nb:
                                   1024 - 128 * mb + 512 * (nb + 1)],
                            start=False, stop=True)
                dest = t_scr[:] if e5slot is None else t_e5[:, mb, e5slot, :]
                nc.scalar.activation(dest, ps[:], AF.Exp, scale=0.15625,
                                     accum_out=rp_st[:, u * 8 + mb:
                                                     u * 8 + mb + 1])

            def stage_full(k):
                for dh in range(2):
                    for h in range(2):
                        head_chain(k, dh, h)
                norm(k)

            def colsums(ci):
                # column sums for e5 slot ci (DoubleRow over mb pairs)
                for nh in range(2):
                    cs = smp.tile([128, 512], F32, name=f"cs{ci}_{nh}",
                                  tag="sm")
                    for jj in range(4):
                        nc.tensor.matmul(
                            cs[:], t_o5[:],
                            t_e5[:, 2 * jj:2 * jj + 2, ci,
                                 512 * nh:512 * (nh + 1)],
                            start=(jj == 0), stop=(jj == 3), perf_mode=DR)
                    nc.vector.tensor_copy(
                        cp_st[0:1, 1024 * ci + 512 * nh:
                              1024 * ci + 512 * (nh + 1)], cs[0:1, :])

            def emit_unit(u, mb):
                unit(u, *UNITS[u][:2], UNITS[u][2], mb)
                if mb == 7 and UNITS[u][2] is not None:
                    colsums(UNITS[u][2])

            stage_full(0)
            for k in range(1, 4):
                pu = k - 1
                emit_unit(pu, 0)
                emit_unit(pu, 1)
                head_chain(k, 0, 0)
                emit_unit(pu, 2)
                emit_unit(pu, 3)
                head_chain(k, 0, 1)
                emit_unit(pu, 4)
                emit_unit(pu, 5)
                head_chain(k, 1, 0)
                emit_unit(pu, 6)
                emit_unit(pu, 7)
                head_chain(k, 1, 1)
                norm(k)
            for mb in range(8):
                emit_unit(3, mb)
            for mb in range(8):
                emit_unit(4, mb)

            # pos: bf16 product of t_h slabs 0 and 3, ones-matmul, r-scales
            t_pp = wk.tile([128, 2, 1024], BF16, name="t_pp", tag="sq")
            nc.vector.tensor_tensor(t_pp[:], t_h[0][:], t_h[3][:], ALU.mult)
            pr = smp.tile([1, 1024], F32, name="rawdot", tag="sm")
            for nh in range(2):
                for dh in range(2):
                    nc.tensor.matmul(pr[0:1, 512 * nh:512 * (nh + 1)],
                                     t_o1[:],
                                     t_pp[:, dh, 512 * nh:512 * (nh + 1)],
                                     start=(dh == 0), stop=(dh == 1))
            tmp = wk.tile([1, 1024], F32, name="ptmp", tag="nln")
            nc.vector.tensor_tensor(tmp[:], pr[:], t_r_tiles[0][:], ALU.mult)
            nc.vector.tensor_tensor(ps_st[:], tmp[:], t_r_tiles[3][:],
                                    ALU.mult)


            nc.sync.dma_start(o_rp, rp_st[:])
            nc.sync.dma_start(o_cp, cp_st[:])
            nc.sync.dma_start(o_ps, ps_st[:])

    # Keep Exp/Ln selectable only from the single table set that holds both,
    # so the compiler never ping-pongs ACT table loads between exp-only and
    # ln-only sets (1283ns per reload).  Entries stay in place so
    # act_func_set_id indices still match act_info.json.
    import concourse.bacc as bacc_mod
    orig_get = bacc_mod.get_activation_tables

    def _pinned_tables(arch):
        tabs = orig_get(arch)
        AFT = mybir.ActivationFunctionType
        both = [k for k, v in tabs.items() if AFT.Exp in v and AFT.Ln in v]
        if not both:
            return tabs
        keep = both[0]
        out = {}
        for k, v in tabs.items():
            if k == keep:
                out[k] = v
            else:
                out[k] = {f for f in v if f not in (AFT.Exp, AFT.Ln)}
        return out

    bacc_mod.get_activation_tables = _pinned_tables
    try:
        nc.compile()
    finally:
        bacc_mod.get_activation_tables = orig_get
    _CACHE["nc"] = nc
    return nc


def _host_inputs(embedded_data, W, b):
    E4np = ml_dtypes.float8_e4m3
    E5np = ml_dtypes.float8_e5m2
    emb = np.asarray(embedded_data, dtype=np.float32)
    embT8 = np.ascontiguousarray(emb.T).astype(E4np)      # [2048, 8192]
    W8 = (np.asarray(W, dtype=np.float32) * 64.0).astype(E4np)
    b64 = (np.asarray(b, dtype=np.float32) * 64.0).astype(np.float32)
    o1 = np.full((128, 1), 1.0 / 64.0, ml_dtypes.bfloat16)
    o5 = np.ones((128, 256), E5np)
    mask = np.zeros((128, 2048), E4np)
    mask[np.arange(128), np.arange(128) + 1024] = -4.0
    ident = (240.0 * np.eye(128, dtype=np.float32)).astype(E4np)
    in_maps = []
    for c in range(8):
        cols = np.concatenate(
            [embT8[:, 1024 * s:1024 * (s + 1)] for s in SLOTS[c]], axis=1)
        in_maps.append({"embT8": np.ascontiguousarray(cols), "W8": W8,
                        "b64": b64, "onesbf": o1, "ones5": o5, "mask": mask,
                        "ident": ident})
    return in_maps


def _combine(results):
    neg = np.zeros(8192, np.float64)
    pos = np.zeros(8192, np.float64)
    for c in range(8):
        S = SLOTS[c]
        rp = results[c]["rowpart"].astype(np.float64)     # [128, 40]
        cp = results[c]["colpart"].astype(np.float64).ravel()
        ps = results[c]["possim"].astype(np.float64)
        sl = [np.s_[1024 * s:1024 * (s + 1)] for s in S]
        for u, (astat, _, _) in enumerate(UNITS):
            if u == 3 and c >= 4:
                continue                                   # diff-4 dedup
            dst = 1024 * S[astat]
            for mb in range(8):
                neg[dst + 128 * mb:dst + 128 * (mb + 1)] += rp[:, 8 * u + mb]
        neg[sl[1]] += cp[0:1024]
        neg[sl[2]] += cp[1024:2048]
        if c < 4:
            neg[sl[3]] += cp[2048:3072]
        neg[sl[3]] += cp[3072:4096]
        if c < 4:
            possim = ps.ravel()
            pos[sl[0]] = possim
            pos[sl[3]] = possim
    loss = -np.mean(10.0 * pos - np.log(neg))
    return np.float32(loss)


def run(embedded_data, W, b, trace=False):
    from concourse import bass_utils
    nc = _build()
    in_maps = _host_inputs(embedded_data, W, b)
    res = bass_utils.run_bass_kernel_spmd(nc, in_maps, core_ids=list(range(8)),
                                          trace=trace)
    return _combine(res.results), res


def kernel(embedded_data, W, b):
    loss, _ = run(embedded_data, W, b, trace=False)
    return np.asarray(loss, dtype=np.float32)


# revision 19
# speedup vs baseline: 1.1887x; 1.1167x over previous
"""NT-Xent contrastive loss on 8 Trainium2 NeuronCores (Bass/Tile), fp8.

Strategy (no collectives; ncfw collective latency floor ~85us):
  * Host casts embT to fp8e4 [2048, 8192] (sigma=1 fits e4m3) and W*64 to
    fp8e4; b*64 stays f32.  Slab cover: core c loads the 4 column-slabs
    S_c = {c, c+1, c+2, c+4} (mod 8) of embT (8.4 MB/core).  Every slab
    pair meets on some core (Z8 difference cover), so each distinct
    1024x1024 block of the 8192x8192 similarity matrix is computed once
    globally (the diff-4 block is deduped on host: cores 0-3 win).
  * Per core: head matmul in fp8 DoubleRow (K=256/instr, 0.5 cyc/row)
    -> h' = 64h in psum -> bias-add copy to bf16 (Pool dh0 / DVE dh1).
    L2 norm: nsq via bf16 ones-matmul into a [33,512] psum tile (rows 0
    and 32), then r = exp(-0.5*ln(nsq) + ln8) on ACT (ln+exp share one
    activation table with the sim exp => zero table reloads), broadcast
    down partitions with gpsimd partition_broadcast, t_on = h*r in fp8e4
    (= 8 * normalized out).
  * 5 sim blocks/core (diag + 4 pairs): one DoubleRow matmul per
    [128,1024] psum tile; diag killed pre-exp with an additive -1e9
    shifted mask (DVE); ACT exp(0.15625*x) with fused row-sum accum
    writes fp8e5 exp values; column sums via DoubleRow ones-matmul over
    mb-pair-interleaved e5 tiles at the end.
  * pos: bf16 product of t_h slabs 0,3 + ones-matmul + r-scales; host
    divides by 64.  Host combine in fp64.
"""
import math
import numpy as np
import ml_dtypes

SLOTS = [(c, (c + 1) % 8, (c + 2) % 8, (c + 4) % 8) for c in range(8)]
# sim units: (stationary slot, moving slot, e5 colsum slot or None)
UNITS = [(0, 0, None), (0, 1, 0), (0, 2, 1), (0, 3, 2), (1, 3, 3)]
LN8 = math.log(8.0)

_CACHE = {}


def _build():
    if "nc" in _CACHE:
        return _CACHE["nc"]
    import concourse.bacc as bacc
    import concourse.tile as tile
    import concourse.mybir as mybir

    F32 = mybir.dt.float32
    BF16 = mybir.dt.bfloat16
    E4 = mybir.dt.float8e4
    E5 = mybir.dt.float8e5
    AF = mybir.ActivationFunctionType
    ALU = mybir.AluOpType
    DR = mybir.MatmulPerfMode.DoubleRow

    nc = bacc.Bacc("TRN2", num_devices=8, debug=False)
    a_emb = nc.dram_tensor("embT8", [2048, 4096], E4, kind="ExternalInput").ap()
    a_W = nc.dram_tensor("W8", [2048, 256], E4, kind="ExternalInput").ap()
    a_b = nc.dram_tensor("b64", [256], F32, kind="ExternalInput").ap()
    a_o1 = nc.dram_tensor("onesbf", [128, 1], BF16, kind="ExternalInput").ap()
    a_o5 = nc.dram_tensor("ones5", [128, 256], E5, kind="ExternalInput").ap()
    a_mask = nc.dram_tensor("mask", [128, 2048], F32, kind="ExternalInput").ap()
    o_rp = nc.dram_tensor("rowpart", [128, 40], F32, kind="ExternalOutput").ap()
    o_cp = nc.dram_tensor("colpart", [1, 4096], F32, kind="ExternalOutput").ap()
    o_ps = nc.dram_tensor("possim", [1, 1024], F32, kind="ExternalOutput").ap()

    with tile.TileContext(nc) as tc:
        with tc.tile_pool(name="sb", bufs=1) as sb, \
             tc.tile_pool(name="wk", bufs=2) as wk, \
             tc.tile_pool(name="hp", bufs=2, space="PSUM") as hp, \
             tc.tile_pool(name="simp", bufs=2, space="PSUM") as simp, \
             tc.tile_pool(name="smp", bufs=1, space="PSUM") as smp:

            # ---- persistent tiles + prologue DMAs.  Critical path first:
            # t_W then stage-0 emb tiles on the sync queue; everything else
            # (consts, stages 1-3) on the gpsimd queue in parallel.
            t_W = sb.tile([128, 8, 2, 2, 128], E4, name="t_W")
            nc.sync.dma_start(
                t_W[:],
                a_W.rearrange("(kk j p) (dh f) -> p kk j dh f",
                              kk=8, j=2, p=128, dh=2, f=128))
            t_e8 = [[None] * 8 for _ in range(4)]
            def load_emb(k, kk, eng):
                t = sb.tile([128, 2, 1024], E4, name=f"t_e8_{k}_{kk}")
                esrc = a_emb[256 * kk:256 * (kk + 1),
                             1024 * k:1024 * (k + 1)]
                eng.dma_start(t[:], esrc.rearrange("(j p) s -> p j s",
                                                   j=2, p=128))
                t_e8[k][kk] = t
            for kk in range(8):
                load_emb(0, kk, nc.sync)
            t_b = sb.tile([128, 2], F32, name="t_b")
            nc.sync.dma_start(t_b[:], a_b.rearrange("(dh p) -> p dh",
                                                    p=128))
            t_o1 = sb.tile([128, 1], BF16, name="t_o1")
            nc.sync.dma_start(t_o1[:], a_o1[:])
            t_o5 = sb.tile([128, 2, 128], E5, name="t_o5")
            nc.sync.dma_start(t_o5[:], a_o5.rearrange("p (j f) -> p j f",
                                                      j=2, f=128))
            t_mask = sb.tile([128, 2048], F32, name="t_mask")
            nc.sync.dma_start(t_mask[:], a_mask[:])
            for kk in range(8):
                load_emb(1, kk, nc.sync)
            for kk in range(8):
                load_emb(2, kk, nc.sync)
            for kk in range(8):
                load_emb(3, kk, nc.sync)

            t_h = [sb.tile([128, 2, 1024], BF16, name=f"t_h{k}")
                   for k in range(4)]
            t_r_tiles = [None] * 4
            t_on = sb.tile([128, 2, 4, 1024], E4, name="t_on")
            t_e5 = sb.tile([128, 8, 4, 1024], E5, name="t_e5")
            t_scr = sb.tile([128, 1024], E5, name="t_scr")
            rp_st = sb.tile([128, 40], F32, name="rp_st")
            cp_st = sb.tile([1, 4096], F32, name="cp_st")
            ps_st = sb.tile([1, 1024], F32, name="ps_st")

            def head_chain(k, dh, h):
                H = hp.tile([128, 512], F32, name=f"H{k}_{dh}_{h}", tag="H")
                for kk in range(8):
                    nc.tensor.matmul(
                        H[:], t_W[:, kk, :, dh, :],
                        t_e8[k][kk][:, :, 512 * h:512 * (h + 1)],
                        start=(kk == 0), stop=(kk == 7), perf_mode=DR)
                nc.vector.tensor_scalar_add(
                    t_h[k][:, dh, 512 * h:512 * (h + 1)], H[:],
                    t_b[:, dh:dh + 1])

            def norm(k):
                t_sq = wk.tile([128, 2, 1024], BF16, name="t_sq", tag="sq")
                nc.vector.tensor_tensor(t_sq[:], t_h[k][:], t_h[k][:],
                                        ALU.mult)
                # t_o1 holds 1/64, so nsq psum = nsq'/64 and
                # exp(-0.5*ln(x)) = 8/sqrt(nsq') -- no activation bias needed
                r_bc = wk.tile([128, 1024], F32, name="r_bc", tag="rbc")
                nsq = smp.tile([1, 1024], F32, name=f"nsq{k}", tag="sm")
                for nh in range(2):
                    for dh in range(2):
                        nc.tensor.matmul(
                            nsq[0:1, 512 * nh:512 * (nh + 1)], t_o1[:],
                            t_sq[:, dh, 512 * nh:512 * (nh + 1)],
                            start=(dh == 0), stop=(dh == 1))
                nln = wk.tile([1, 1024], F32, name="nln", tag="nln")
                nc.scalar.activation(nln[:], nsq[:], AF.Ln)
                t_rk = sb.tile([1, 1024], F32, name=f"t_r{k}")
                t_r_tiles[k] = t_rk
                nc.scalar.activation(t_rk[:], nln[:], AF.Exp, scale=-0.5)
                for nh in range(2):
                    nc.gpsimd.partition_broadcast(
                        r_bc[:, 512 * nh:512 * (nh + 1)],
                        t_rk[0:1, 512 * nh:512 * (nh + 1)])
                for dh in range(2):
                    nc.vector.tensor_tensor(t_on[:, dh, k, :],
                                            t_h[k][:, dh, :], r_bc[:],
                                            ALU.mult)

            def unit(u, a, b, e5slot, mb):
                ps = simp.tile([128, 1024], F32, name="ps", tag="ps")
                for nb in range(2):
                    nc.tensor.matmul(ps[:, 512 * nb:512 * (nb + 1)],
                                     t_on[:, :, a, 128 * mb:128 * (mb + 1)],
                                     t_on[:, :, b, 512 * nb:512 * (nb + 1)],
                                     start=True, stop=True, perf_mode=DR)
                if a == b:
                    nc.vector.tensor_tensor(
                        ps[:], ps[:],
                        t_mask[:, 1024 - 128 * mb:2048 - 128 * mb], ALU.add)
                dest = t_scr[:] if e5slot is None else t_e5[:, mb, e5slot, :]
                nc.scalar.activation(dest, ps[:], AF.Exp, scale=0.15625,
                                     accum_out=rp_st[:, u * 8 + mb:
                                                     u * 8 + mb + 1])

            def stage_full(k):
                for dh in range(2):
                    for h in range(2):
                        head_chain(k, dh, h)
                norm(k)

            def colsums(ci):
                # column sums for e5 slot ci (DoubleRow over mb pairs)
                for nh in range(2):
                    cs = smp.tile([128, 512], F32, name=f"cs{ci}_{nh}",
                                  tag="sm")
                    for jj in range(4):
                        nc.tensor.matmul(
                            cs[:], t_o5[:],
                            t_e5[:, 2 * jj:2 * jj + 2, ci,
                                 512 * nh:512 * (nh + 1)],
                            start=(jj == 0), stop=(jj == 3), perf_mode=DR)
                    nc.vector.tensor_copy(
                        cp_st[0:1, 1024 * ci + 512 * nh:
                              1024 * ci + 512 * (nh + 1)], cs[0:1, :])

            def emit_unit(u, mb):
                unit(u, *UNITS[u][:2], UNITS[u][2], mb)

            stage_full(0)
            for k in range(1, 4):
                pu = k - 1
                emit_unit(pu, 0)
                emit_unit(pu, 1)
                head_chain(k, 0, 0)
                emit_unit(pu, 2)
                emit_unit(pu, 3)
                head_chain(k, 0, 1)
                emit_unit(pu, 4)
                emit_unit(pu, 5)
                head_chain(k, 1, 0)
                emit_unit(pu, 6)
                emit_unit(pu, 7)
                head_chain(k, 1, 1)
                norm(k)
            for mb in range(8):
                emit_unit(3, mb)
            for mb in range(8):
                emit_unit(4, mb)
            for ci in range(4):
                colsums(ci)

            # pos: bf16 product of t_h slabs 0 and 3, ones-matmul, r-scales
            t_pp = wk.tile([128, 2, 1024], BF16, name="t_pp", tag="sq")
            nc.vector.tensor_tensor(t_pp[:], t_h[0][:], t_h[3][:], ALU.mult)
            pr = smp.tile([1, 1024], F32, name="rawdot", tag="sm")
            for nh in range(2):
                for dh in range(2):
                    nc.tensor.matmul(pr[0:1, 512 * nh:512 * (nh + 1)],
                                     t_o1[:],
                                     t_pp[:, dh, 512 * nh:512 * (nh + 1)],
                                     start=(dh == 0), stop=(dh == 1))
            tmp = wk.tile([1, 1024], F32, name="ptmp", tag="nln")
            nc.vector.tensor_tensor(tmp[:], pr[:], t_r_tiles[0][:], ALU.mult)
            nc.vector.tensor_tensor(ps_st[:], tmp[:], t_r_tiles[3][:],
                                    ALU.mult)


            nc.sync.dma_start(o_rp, rp_st[:])
            nc.sync.dma_start(o_cp, cp_st[:])
            nc.sync.dma_start(o_ps, ps_st[:])

    # Keep Exp/Ln selectable only from the single table set that holds both,
    # so the compiler never ping-pongs ACT table loads between exp-only and
    # ln-only sets (1283ns per reload).  Entries stay in place so
    # act_func_set_id indices still match act_info.json.
    import concourse.bacc as bacc_mod
    orig_get = bacc_mod.get_activation_tables

    def _pinned_tables(arch):
        tabs = orig_get(arch)
        AFT = mybir.ActivationFunctionType
        both = [k for k, v in tabs.items() if AFT.Exp in v and AFT.Ln in v]
        if not both:
            return tabs
        keep = both[0]
        out = {}
        for k, v in tabs.items():
            if k == keep:
                out[k] = v
            else:
                out[k] = {f for f in v if f not in (AFT.Exp, AFT.Ln)}
        return out

    bacc_mod.get_activation_tables = _pinned_tables
    try:
        nc.compile()
    finally:
        bacc_mod.get_activation_tables = orig_get
    _CACHE["nc"] = nc
    return nc


def _host_inputs(embedded_data, W, b):
    E4np = ml_dtypes.float8_e4m3
    E5np = ml_dtypes.float8_e5m2
    emb = np.asarray(embedded_data, dtype=np.float32)
    embT8 = np.ascontiguousarray(emb.T).astype(E4np)      # [2048, 8192]
    W8 = (np.asarray(W, dtype=np.float32) * 64.0).astype(E4np)
    b64 = (np.asarray(b, dtype=np.float32) * 64.0).astype(np.float32)
    o1 = np.full((128, 1), 1.0 / 64.0, ml_dtypes.bfloat16)
    o5 = np.ones((128, 256), E5np)
    mask = np.zeros((128, 2048), np.float32)
    mask[np.arange(128), np.arange(128) + 1024] = -1e9
    in_maps = []
    for c in range(8):
        cols = np.concatenate(
            [embT8[:, 1024 * s:1024 * (s + 1)] for s in SLOTS[c]], axis=1)
        in_maps.append({"embT8": np.ascontiguousarray(cols), "W8": W8,
                        "b64": b64, "onesbf": o1, "ones5": o5, "mask": mask})
    return in_maps


def _combine(results):
    neg = np.zeros(8192, np.float64)
    pos = np.zeros(8192, np.float64)
    for c in range(8):
        S = SLOTS[c]
        rp = results[c]["rowpart"].astype(np.float64)     # [128, 40]
        cp = results[c]["colpart"].astype(np.float64).ravel()
        ps = results[c]["possim"].astype(np.float64)
        sl = [np.s_[1024 * s:1024 * (s + 1)] for s in S]
        for u, (astat, _, _) in enumerate(UNITS):
            if u == 3 and c >= 4:
                continue                                   # diff-4 dedup
            dst = 1024 * S[astat]
            for mb in range(8):
                neg[dst + 128 * mb:dst + 128 * (mb + 1)] += rp[:, 8 * u + mb]
        neg[sl[1]] += cp[0:1024]
        neg[sl[2]] += cp[1024:2048]
        if c < 4:
            neg[sl[3]] += cp[2048:3072]
        neg[sl[3]] += cp[3072:4096]
        if c < 4:
            possim = ps.ravel()
            pos[sl[0]] = possim
            pos[sl[3]] = possim
    loss = -np.mean(10.0 * pos - np.log(neg))
    return np.float32(loss)


def run(embedded_data, W, b, trace=False):
    from concourse import bass_utils
    nc = _build()
    in_maps = _host_inputs(embedded_data, W, b)
    res = bass_utils.run_bass_kernel_spmd(nc, in_maps, core_ids=list(range(8)),
                                          trace=trace)
    return _combine(res.results), res


def kernel(embedded_data, W, b):
    loss, _ = run(embedded_data, W, b, trace=False)
    return np.asarray(loss, dtype=np.float32)
